# revision 71
# baseline (speedup 1.0000x reference)
"""Trainium2 Bass kernel for the sparse video-attention module.

Model (reference):
    k = conv3x3(x[:, 0], w_k)                     # key from first frame only
    q = conv3x3(x, w_q); v = conv3x3(x, w_v)      # per-frame
    dots[b,t,h,w] = sum_c q[b,t,c,h,w] * k[b,c,h,w]
    attn = softmax_T(dots)
    pooled = sum_t attn[...,t] * v[...,t]         # (B, DH, H, W)
    out = conv3x3(pooled, w_out) + b_out          # identical for every t

Sharding: 8 cores = (batch b in 0..3) x (row half in 0..1). Each core owns 32
output rows of one batch element; all coupling (softmax over T, convs) is
local given the row halo, so there is no inter-core communication. Each core
computes exactly 33 attention rows: its 32 plus the ONE real halo row toward
the seam -- odd cores get their x slice (and conv weights) vertically
flipped host-side so the real halo is always local row 32, and the
out-of-image halo row (pure zeros) is never computed at all. The host
pre-pads/slices inputs per core and re-assembles (un-flipping odd halves) +
broadcasts the output over T at the end.

Precision/engine strategy: all 17 q/k/v conv frames run as fp8e4m3
DoubleRow matmuls (0.5 PE cycles/row, contraction 2x128 channels per
call) with error compensation: x and w are each split hi+lo on the e4m3
grid (host-side) and the conv accumulates xh*wh + xh*wl + xl*wh in PSUM,
dropping only the O(u^2) xl*wl term -- ~0.2% relative error per conv at
3/4 the f32r cycle cost (measured 1.6e-2 final max-rel-err vs the 2e-2
gate; the deterministic inputs make this a fixed, not statistical,
margin). w is pre-scaled by 256 to keep its lo-split out of e4m3
subnormals; the 1/256^2 descale of the q*k product is folded into the
dots ones-vector, and attn rows are scaled by 1/256 via the softmax
normalizer to cancel the 256x the v PSUM carries. The out conv on pooled
stays f32r. A short chain of throwaway ident transposes warms the PE
p-state ramp during the initial DMA wait.

Per-core layout: pixels of the 33 compute rows are flattened row-major into
2112 positions (staged into a zero-padded 17*128 grid for the dots
transposes). Convs are matmuls with C on the contraction dim and 9 spatial
taps accumulated in PSUM; spatial shifts are plain access-pattern offsets
into a zero-padded [128, 2, 35, 66] SBUF image (ktile dim = channel group
for DoubleRow).

Two passes over frames: q-pass computes dots[t, pix] via ones-stationary
M=1 f32r matmuls over qk, PE-transposed to [pixel, t] so the softmax over
T is a free-dim reduction. The v-pass then convolves each frame and
accumulates attn_t * v_t straight from the conv PSUM banks on the Vector
engine, hidden under the next frame's matmuls; attn rows (bf16) reach all
128 partitions via one DRAM-bounced stride-0 broadcast DMA per frame
(frame 0's v conv is drained to SBUF by the Act engine so its PSUM banks
recycle without waiting on that bounce). The final accumulation add
writes a float32r copy of pooled so the out-conv runs at full PE rate,
with the last v conv emitted row-block-outer so the out conv chases the
accumulation chain instead of trailing it.
"""

import sys

import numpy as np

for _p in ("/opt/trn_rl_repo", "/root/.axon_site/_ro/trn_rl_repo"):
    if _p not in sys.path:
        sys.path.insert(0, _p)

import ml_dtypes

E4 = ml_dtypes.float8_e4m3

B, T, C, H, W = 4, 8, 256, 64, 64
DH = 128
NCORES = 8
CR = 33            # compute rows per core (32 own + 1 real halo at the seam)
XR = 35            # x rows per core (locals -1..33: compute rows + conv halo)
WP = W + 2         # zero-padded width
NPIX = CR * W      # 2112 pixels per core
NBLK = 17          # 128-pixel transpose blocks (last one half zero-padded)
SPIX = NBLK * 128  # 2176-wide staging grid for dots/attn transposes
SW = 256.0         # fp8 weight pre-scale (keeps w-lo out of e4m3 subnormals)
# Row blocks over the 33 compute rows. Free dims 448/448/448/384/384 -- all
# >=256 so f32r matmuls run 1 cycle/row, and each fits one PSUM bank.
RB = [(0, 7), (7, 7), (14, 7), (21, 6), (27, 6)]
OUT_RB = [(0, 8), (8, 8), (16, 8), (24, 8)]  # over the 32 output rows

WARMUP_BIG = 12    # PE-ramp warm-up transposes (256-cycle) during DMA lead-in
WARMUP_SMALL = 8   # fine-grained (64-cycle) tail of the warm-up chain

RUN_KWARGS: dict = {}   # extra kwargs for run_bass_kernel_spmd (test hook)
LAST_RESULT = None      # last BassKernelResults (test hook)

_cache: dict = {}


def _build_nc():
    from contextlib import ExitStack

    import concourse.mybir as mybir
    import concourse.tile as tile
    from concourse import bacc
    from concourse.masks import make_identity

    f32 = mybir.dt.float32
    f32r = mybir.dt.float32r
    f8 = mybir.dt.float8e4
    bf16 = mybir.dt.bfloat16
    DR = mybir.MatmulPerfMode.DoubleRow
    AF = mybir.ActivationFunctionType
    X = mybir.AxisListType.X

    nc = bacc.Bacc("TRN2", target_bir_lowering=False)

    xh_d = nc.declare_dram_parameter("xh", [T, 128, 2 * XR * WP], f8, isOutput=False)
    xl_d = nc.declare_dram_parameter("xl", [T, 128, 2 * XR * WP], f8, isOutput=False)
    # duplicate of frame-0 xh rows 0..15: its own tile, so the k conv's
    # leading matmuls aren't gated on the whole frame-0 load (tile
    # dependency tracking is whole-tile, not subtile)
    xp_d = nc.declare_dram_parameter("xp", [128, 2 * 16 * WP], f8, isOutput=False)
    wqh_d = nc.declare_dram_parameter("wqh", [128, 2, 9, 128], f8, isOutput=False)
    wql_d = nc.declare_dram_parameter("wql", [128, 2, 9, 128], f8, isOutput=False)
    wvh_d = nc.declare_dram_parameter("wvh", [128, 2, 9, 128], f8, isOutput=False)
    wvl_d = nc.declare_dram_parameter("wvl", [128, 2, 9, 128], f8, isOutput=False)
    wkh_d = nc.declare_dram_parameter("wkh", [128, 2, 9, 128], f8, isOutput=False)
    wkl_d = nc.declare_dram_parameter("wkl", [128, 2, 9, 128], f8, isOutput=False)
    wo_d = nc.declare_dram_parameter("wo", [128, 9, 256], f32r, isOutput=False)
    bo_d = nc.declare_dram_parameter("bo", [128, 2], f32, isOutput=False)
    out_d = nc.declare_dram_parameter("out", [2, 128, 32 * W], f32, isOutput=True)

    with tile.TileContext(nc) as tc, ExitStack() as ctx:
        singles = ctx.enter_context(tc.tile_pool(name="singles", bufs=1))
        xpool = ctx.enter_context(tc.tile_pool(name="xpool", bufs=5))
        sb = ctx.enter_context(tc.tile_pool(name="sb", bufs=1))
        sm = ctx.enter_context(tc.tile_pool(name="sm", bufs=2))
        qkpool = ctx.enter_context(tc.tile_pool(name="qkpool", bufs=2))
        abpool = ctx.enter_context(tc.tile_pool(name="abpool", bufs=2))
        dtpool = ctx.enter_context(tc.tile_pool(name="dtpool", bufs=2))

        def load_x8(t, split=False):
            # hi/lo e4m3 frame tiles [128, 2(group=ktile), XR*WP]; one DMA
            # per tensor (the per-DMA SP descriptor-gen cost dominates).
            tiles = []
            flats = []
            for tag, dram in (("xh", xh_d), ("xl", xl_d)):
                xt = xpool.tile([128, 2, XR * WP], f8, tag=tag, name=f"{tag}{t}")
                flats.append((xt.rearrange("p g l -> p (g l)"), dram))
                tiles.append(xt.rearrange("p g (r c) -> p g r c", c=WP))
            if split:
                return tiles, flats
            for flat, dram in flats:
                nc.sync.dma_start(out=flat, in_=dram[t])
            return tiles

        # k-conv operands first (wkh + frame-0 x), so the PE starts as early
        # as possible under the serial DMA pipe.
        wkh_sb = singles.tile([128, 2, 9, 128], f8, tag="wkh")
        wkl_sb = singles.tile([128, 2, 9, 128], f8, tag="wkl")
        nc.sync.dma_start(out=wkh_sb, in_=wkh_d[:])
        # frame 0 arrives in k-conv consumption order: the rows-0..15 prefix
        # first (feeds the front-run matmuls), then the full frame, then wkl
        # and xl -- needed only from the conv's second/third term on.
        xp0 = singles.tile([128, 2, 16, WP], f8, tag="xp0")
        nc.sync.dma_start(out=xp0.rearrange("p g r c -> p (g r c)"), in_=xp_d[:])
        x8_0, _fl0 = load_x8(0, split=True)
        _xh3 = _fl0[0][0].rearrange("p (g l) -> p g l", g=2)
        _sh3 = _fl0[0][1][0].rearrange("p (g l) -> p g l", g=2)
        L0 = XR * WP
        for h in range(2):
            cols = slice(h * L0 // 2, (h + 1) * L0 // 2)
            nc.sync.dma_start(out=_xh3[:, :, cols], in_=_sh3[:, :, cols])
        nc.sync.dma_start(out=wkl_sb, in_=wkl_d[:])
        nc.sync.dma_start(out=_fl0[1][0], in_=_fl0[1][1][0])
        wqh_sb = singles.tile([128, 2, 9, 128], f8, tag="wqh")
        wql_sb = singles.tile([128, 2, 9, 128], f8, tag="wql")
        wvh_sb = singles.tile([128, 2, 9, 128], f8, tag="wvh")
        wvl_sb = singles.tile([128, 2, 9, 128], f8, tag="wvl")
        wo_sb = singles.tile([128, 9, 256], f32r, tag="wo")
        bo_sb = singles.tile([128, 2], f32, tag="bo")
        nc.sync.dma_start(out=bo_sb, in_=bo_d[:])
        nc.sync.dma_start(out=wqh_sb, in_=wqh_d[:])
        nc.sync.dma_start(out=wql_sb, in_=wql_d[:])
        nc.sync.dma_start(out=wvh_sb, in_=wvh_d[:])
        nc.sync.dma_start(out=wvl_sb, in_=wvl_d[:])
        nc.sync.dma_start(out=wo_sb, in_=wo_d[:])

        # The PE warm-up chain reads ident BEFORE it is initialized (the
        # transposed garbage lands in a PSUM region that is start=True
        # overwritten later): the memset + affine_select that build the real
        # identity run after the warm-up reads, still long before the first
        # true transpose use at the dots gather.
        ident = singles.tile([128, 128], f32, tag="ident")
        ident_bf = singles.tile([128, 128], bf16, tag="ident_bf")
        ones_col = singles.tile([128, 1], f32, tag="ones")
        # q and k PSUMs both carry the SW weight scale; the ones column
        # descales their product in the dots reduction
        nc.vector.memset(ones_col, 1.0 / (SW * SW))
        ones_r = singles.tile([128, 1], f32r, tag="ones_r")
        nc.vector.tensor_copy(ones_r, ones_col)
        csw = singles.tile([128, 1], f32, tag="csw")
        nc.vector.memset(csw, 1.0 / SW)       # attn descale (cancels v scale)
        eps_sb = singles.tile([128, 1], f32, tag="eps")
        nc.vector.memset(eps_sb, 1e-30)

        def conv3x3_f8(psums, x8, wh_sb, wl_sb, full=False, r_outer=False,
                       schedule=None):
            # compensated fp8 conv: xh*wh + xh*wl + xl*wh (+ xl*wl when
            # full=True), DoubleRow over both channel groups (ktile dim) in
            # each call. r_outer finishes PSUM banks in row-block order so
            # downstream per-block consumers can chase the conv; schedule
            # overrides the (term, tap, block) emission order entirely.
            xh4, xl4 = x8
            terms = [(xh4, wh_sb), (xh4, wl_sb), (xl4, wh_sb)]
            if full:
                terms.append((xl4, wl_sb))
            nt = len(terms)

            def mm(ti, j, r):
                xt4, w_sb = terms[ti]
                ky, kx = divmod(j, 3)
                R0, nr = RB[r]
                nc.tensor.matmul(
                    psums[r][:, : nr * W],
                    w_sb[:, :, j, :],
                    xt4[:, :, R0 + ky : R0 + ky + nr, kx : kx + W],
                    start=(ti == 0 and j == 0),
                    stop=(ti == nt - 1 and j == 8),
                    perf_mode=DR,
                )

            if schedule is None:
                if r_outer:
                    schedule = [
                        (ti, j, r)
                        for r in range(len(RB))
                        for ti in range(nt)
                        for j in range(9)
                    ]
                else:
                    schedule = [
                        (ti, j, r)
                        for ti in range(nt)
                        for j in range(9)
                        for r in range(len(RB))
                    ]
            for ti, j, r in schedule:
                mm(ti, j, r)

        dpool = ctx.enter_context(tc.tile_pool(name="dpool", bufs=1, space="DRAM"))
        attnT_dram = dpool.tile([8, SPIX], bf16, tag="attnTd")

        k_sb = sb.tile([128, NPIX], f32, tag="k")
        attnT_sb = sb.tile([8, SPIX], bf16, tag="attnT")
        # pooled rows 0..33 = locals -1..32; row 0 is the zero out-of-image
        # halo (memset once), computed locals land at tile row +1
        pooled = sb.tile([128, CR + 1, WP], f32, tag="pooled")
        pooled_r = sb.tile([128, CR + 1, WP], f32r, tag="pooled_r")
        out_sb = sb.tile([128, 2, 32 * W], f32, tag="out")

        psc = ctx.enter_context(tc.tile_pool(name="psc", bufs=6, space="PSUM"))

        def vconv(t, x8):
            vps = [
                psc.tile([128, 512], f32, tag="cv", name=f"vps{t}_{r}")
                for r in range(len(RB))
            ]
            # the last frame releases its PSUM banks in row-block order so
            # the final vapply chain and the out conv can chase it
            conv3x3_f8(vps, x8, wvh_sb, wvl_sb, r_outer=(t == T - 1))
            return vps

        def vapply(t, vsrc):
            # pooled += attn_t (broadcast over channels) * v_t; vsrc(r) gives
            # the v rows for block r (conv PSUM bank, or SBUF for frame 0).
            # attn rows carry 1/SW so the product is true-scale. The last
            # frame's add writes the f32r copy the out-conv consumes.
            # One whole-frame broadcast DMA: per-DMA SP descriptor-gen
            # (~650ns) dominates small transfers.
            abf = abpool.tile([128, SPIX], bf16, tag="ab", name=f"ab{t}")
            nc.sync.dma_start(
                out=abf,
                in_=attnT_dram[t : t + 1, :].to_broadcast((128, SPIX)),
            )
            for r, (R0, nr) in enumerate(RB):
                rows = slice(R0 + 1, R0 + 1 + nr)
                cols = slice(R0 * W, (R0 + nr) * W)
                ab = abf[:, R0 * W :]
                if t == 0:
                    nc.vector.tensor_mul(
                        pooled[:, rows, 1 : W + 1],
                        vsrc(r).rearrange("p (r c) -> p r c", c=W),
                        ab[:, : nr * W].rearrange("p (r c) -> p r c", c=W),
                    )
                    continue
                u = qkpool.tile([128, NPIX], f32, tag="qk", name=f"u{t}_{r}")
                nc.vector.tensor_mul(u[:, cols], vsrc(r), ab[:, : nr * W])
                dst = pooled_r if t == T - 1 else pooled
                nc.vector.tensor_add(
                    dst[:, rows, 1 : W + 1],
                    pooled[:, rows, 1 : W + 1],
                    u[:, cols].rearrange("p (r c) -> p r c", c=W),
                )

        with (
            tc.tile_pool(name="psd", bufs=1, space="PSUM") as psd,
            tc.tile_pool(name="psdd", bufs=1, space="PSUM") as psdd,
        ):
            dots_ps = psd.tile([128, NBLK * 8], f32, tag="dots")
            # PE warm-up: the cost model ramps the PE clock (0.65 -> 1.2 ->
            # 2.4 GHz) over the first ~3us of continuous busyness. Burn the
            # ramp on throwaway ident transposes during the initial DMA wait
            # instead of on the k conv. Tuned to drain right as x0 lands.
            for i in range(WARMUP_BIG):
                nc.tensor.transpose(dots_ps[:, 0:128], ident, ident)
            for i in range(WARMUP_SMALL):
                nc.tensor.transpose(
                    dots_ps[:32, 0:32], ident[:32, :32], ident[:32, :32]
                )
            make_identity(nc, ident)
            nc.vector.tensor_copy(ident_bf, ident)
            dots_sb = sb.tile([8, SPIX], f32, tag="dsb")
            # the staging grid's zero tail (pixels NPIX..SPIX) keeps the
            # padded transpose block NaN-free
            nc.vector.memset(dots_sb[:, NPIX:], 0.0)

            # ---- phase 1: k = conv(x[0], w_k), fp8 ----
            kps = [
                psc.tile([128, 512], f32, tag="cv", name=f"kps{r}")
                for r in range(len(RB))
            ]
            # front-run: first-term matmuls for row blocks 0-1 read the
            # dedicated rows-0..15 prefix tile, so they start on ~3.6us of
            # DMA instead of waiting out the full frame-0 load
            for j in range(9):
                ky, kx = divmod(j, 3)
                for r in (0, 1):
                    R0, nr = RB[r]
                    nc.tensor.matmul(
                        kps[r][:, : nr * W],
                        wkh_sb[:, :, j, :],
                        xp0[:, :, R0 + ky : R0 + ky + nr, kx : kx + W],
                        start=(j == 0),
                        stop=False,
                        perf_mode=DR,
                    )
            ksched = (
                [(0, j, r) for j in range(9) for r in (2, 3, 4)]
                + [(ti, j, r) for ti in (1, 2) for j in range(9) for r in range(5)]
            )
            conv3x3_f8(kps, x8_0, wkh_sb, wkl_sb, schedule=ksched)
            for r, (R0, nr) in enumerate(RB):
                nc.scalar.activation(
                    k_sb[:, R0 * W : (R0 + nr) * W], kps[r][:, : nr * W], AF.Copy
                )

            # ---- phase 2: per frame q conv (fp8) + dots ----
            for t in range(T):
                x8 = x8_0 if t == 0 else load_x8(t)
                qps = [
                    psc.tile([128, 512], f32, tag="cv", name=f"qps{t}_{r}")
                    for r in range(len(RB))
                ]
                conv3x3_f8(qps, x8, wqh_sb, wql_sb)
                qk = qkpool.tile([128, NPIX], f32r, tag="qk", name=f"qk{t}")
                dtmp = dtpool.tile([1, NPIX], f32, tag="dtmp", name=f"dt{t}")
                for r, (R0, nr) in reversed(list(enumerate(RB))):
                    cols = slice(R0 * W, (R0 + nr) * W)
                    nc.vector.tensor_mul(qk[:, cols], qps[r][:, : nr * W], k_sb[:, cols])
                    # dots[t, pix] = (1/SW^2) * sum_c qk[c, pix]: the ones
                    # column descales the weight scale both convs carry
                    dps = psdd.tile([1, 512], f32, tag="dd", name=f"dd{t}_{r}")
                    nc.tensor.matmul(
                        dps[:, : nr * W],
                        ones_r[:, 0:1],
                        qk[:, cols],
                        start=True,
                        stop=True,
                    )
                    nc.scalar.activation(dtmp[:, cols], dps[:, : nr * W], AF.Copy)
                    if t == T - 1:
                        # last frame: per-block staging DMAs so the gather
                        # transposes (waiting on row 7) release incrementally
                        nc.sync.dma_start(
                            out=dots_sb[t : t + 1, cols], in_=dtmp[:, cols]
                        )
                if t < T - 1:
                    # one DMA per frame into the [t, pix] staging row
                    nc.sync.dma_start(out=dots_sb[t : t + 1, :NPIX], in_=dtmp)

            # gather dots into [pixel, t] layout for the softmax
            for i in range(NBLK):
                nc.tensor.transpose(
                    dots_ps[:, i * 8 : (i + 1) * 8],
                    dots_sb[:, i * 128 : (i + 1) * 128],
                    ident[:8, :8],
                )

            # v conv for frame 0 keeps the PE busy through the softmax below;
            # reuses the still-resident x0 fp8 tiles (no reload). Drain it to
            # SBUF on the idle Act engine right away: vapply(0) sits behind
            # the attn DRAM-bounce, and holding the PSUM banks that long
            # would stall vconv(1). Prefetch the first v-pass frames NOW:
            # DMAs issued later queue behind the attn-dependent attnT
            # transfers (head-of-line) and would stall the v-pass.
            vps0 = vconv(0, x8_0)
            v0_sb = sb.tile([128, NPIX], f32, tag="v0")
            for r, (R0, nr) in enumerate(RB):
                nc.scalar.activation(
                    v0_sb[:, R0 * W : (R0 + nr) * W], vps0[r][:, : nr * W], AF.Copy
                )
            vload = {t: load_x8(t) for t in (1, 2)}

            # ---- softmax over t (free dim) ----
            dots3 = dots_ps.rearrange("p (i t) -> p i t", t=8)
            nmax = sm.tile([128, NBLK], f32, tag="nmax")
            nc.vector.reduce_max(out=nmax, in_=dots3, axis=X, negate=True)
            dm = sm.tile([128, NBLK, 8], f32, tag="dm")
            nc.vector.tensor_add(
                dm, dots3, nmax[:, :, None].to_broadcast((128, NBLK, 8))
            )
            nc.scalar.activation(dm, dm, AF.Exp)
            ssum = sm.tile([128, NBLK], f32, tag="ssum")
            nc.vector.reduce_sum(out=ssum, in_=dm, axis=X)
            nc.scalar.add(ssum, ssum, eps_sb[:])
            rs = sm.tile([128, NBLK], f32, tag="rs")
            nc.vector.reciprocal(rs, ssum)
            # fold the v-path descale into the normalizer: attn rows = true/SW
            nc.vector.tensor_mul(rs, rs, csw[:, 0:1].to_broadcast((128, NBLK)))
            attn = sm.tile([128, NBLK, 8], bf16, tag="attn")
            nc.vector.tensor_mul(
                attn, dm, rs[:, :, None].to_broadcast((128, NBLK, 8))
            )

        # ---- transpose attn to [t, pixel], bounce via DRAM for broadcast ----
        with tc.tile_pool(name="pst", bufs=2, space="PSUM") as pst:
            for ci in range((NBLK + 3) // 4):
                blocks = range(4 * ci, min(4 * ci + 4, NBLK))
                tp = pst.tile([8, 512], bf16, tag="attnT_ps", name=f"tp{ci}")
                for ib, i in enumerate(blocks):
                    nc.tensor.transpose(
                        tp[:, ib * 128 : (ib + 1) * 128], attn[:, i, :], ident_bf
                    )
                n = len(blocks) * 128
                c0 = 4 * ci * 128
                nc.vector.tensor_copy(attnT_sb[:, c0 : c0 + n], tp[:, :n])
            nc.sync.dma_start(out=attnT_dram[:], in_=attnT_sb)

        # ---- phase 3: v convs (fp8) with attn-weighted accumulation ----
        nc.vector.memset(pooled_r[:, 0:1].bitcast(f32), 0.0)
        nc.vector.memset(pooled_r[:, :, 0:1].bitcast(f32), 0.0)
        nc.vector.memset(pooled_r[:, :, W + 1 : W + 2].bitcast(f32), 0.0)
        vapply(0, lambda r: v0_sb[:, RB[r][0] * W : (RB[r][0] + RB[r][1]) * W])
        for t in range(1, T):
            x8 = vload.pop(t, None) or load_x8(t)
            if t + 2 < T and t + 2 not in vload:
                vload[t + 2] = load_x8(t + 2)
            vps = vconv(t, x8)
            vapply(t, lambda r, vps=vps: vps[r][:, : RB[r][1] * W])

        # ---- phase 4: out = conv(pooled, w_out) + b, f32r ----
        with tc.tile_pool(name="pso", bufs=2, space="PSUM") as pso:
            for R0o, nr in OUT_RB:
                for g in range(2):
                    # the very last group is the exit critical path: run it
                    # as two independent row-half pieces so its store chain
                    # pipelines with its matmuls
                    pieces = 2 if (R0o, g) == (OUT_RB[-1][0], 1) else 1
                    pnr = nr // pieces
                    for ci in range(pieces):
                        Rp = R0o + ci * pnr
                        op = pso.tile(
                            [128, 512], f32, tag="out_ps", name=f"op{Rp}_{g}"
                        )
                        for j in range(9):
                            ky, kx = divmod(j, 3)
                            nc.tensor.matmul(
                                op[:, : pnr * W],
                                wo_sb[:, j, g * 128 : (g + 1) * 128],
                                pooled_r[:, Rp + ky : Rp + ky + pnr, kx : kx + W],
                                start=(j == 0),
                                stop=(j == 8),
                            )
                        nc.scalar.add(
                            out_sb[:, g, Rp * W : (Rp + pnr) * W],
                            op[:, : pnr * W],
                            bo_sb[:, g : g + 1],
                        )
                        nc.sync.dma_start(
                            out=out_d[g, :, Rp * W : (Rp + pnr) * W],
                            in_=out_sb[:, g, Rp * W : (Rp + pnr) * W],
                        )

    nc.compile()
    return nc


def _get_nc():
    if "nc" not in _cache:
        _cache["nc"] = _build_nc()
    return _cache["nc"]


def _round_f32r(a):
    """Round fp32 to the FP32r grid (e8m11 in the top 20 bits, RNE)."""
    u = np.ascontiguousarray(a, np.float32).view(np.uint32).copy()
    u += np.uint32(0x7FF) + ((u >> np.uint32(12)) & np.uint32(1))
    u &= np.uint32(0xFFFFF000)
    return u.view(np.float32)


def _split8(a):
    """hi/lo e4m3 split: a ~= hi + lo with ~u^2 residual."""
    a = np.ascontiguousarray(a, np.float32)
    hi = a.astype(E4)
    lo = (a - hi.astype(np.float32)).astype(E4)
    return hi, lo


def _shared_inputs(w_k, w_q, w_v, w_out, b_out):
    def conv_lhst(w, scale=1.0):  # (co=128, ci=256, 3, 3) -> (ci128, g, j, co)
        return np.ascontiguousarray(
            np.asarray(w, np.float32)
            .reshape(128, 2, 128, 3, 3)
            .transpose(2, 1, 3, 4, 0)
            .reshape(128, 2, 9, 128)
        ) * scale

    bo = np.ascontiguousarray(np.asarray(b_out, np.float32).reshape(2, 128).T)

    def one(flip):
        # odd cores see a vertically flipped image: flip the kernel rows too
        def f(w):
            w = np.asarray(w, np.float32)
            return w[:, :, ::-1, :] if flip else w

        wqh, wql = _split8(conv_lhst(f(w_q), SW))
        wvh, wvl = _split8(conv_lhst(f(w_v), SW))
        wkh, wkl = _split8(conv_lhst(f(w_k), SW))
        wo = np.ascontiguousarray(  # (co=256, dh=128, 3, 3) -> (dh, j, co)
            f(w_out).transpose(1, 2, 3, 0).reshape(128, 9, 256)
        )
        return {
            "wqh": wqh, "wql": wql, "wvh": wvh, "wvl": wvl,
            "wkh": wkh, "wkl": wkl,
            "wo": _round_f32r(wo),
            "bo": bo,
        }

    return [one(False), one(True)]


def _x_splits(x):
    xf = np.ascontiguousarray(x, np.float32)
    xh = xf.astype(E4)
    xl = (xf - xh.astype(np.float32)).astype(E4)
    return xh, xl


def core_inputs(c, x, xh, xl, shared):
    b, half = divmod(c, 2)
    # canonical core layout: x tile rows 0..34 = locals -1..33, computed
    # attention rows = locals 0..32 (local 32 is the real seam halo), out
    # rows = locals 0..31. half 0: local == global (tile rows 1..34 <-
    # globals 0..33, row 0 zero). half 1: local L == global 63-L, i.e. the
    # slice is vertically flipped (tile rows 1..34 <- globals 63..30
    # reversed, row 0 = global 64 = zero).
    xph = np.zeros((T, 128, 2, XR, WP), E4)
    xpl = np.zeros((T, 128, 2, XR, WP), E4)
    src_h = xh[b].reshape(T, 2, 128, H, W).transpose(0, 2, 1, 3, 4)
    src_l = xl[b].reshape(T, 2, 128, H, W).transpose(0, 2, 1, 3, 4)
    if half == 0:
        xph[:, :, :, 1:, 1 : W + 1] = src_h[:, :, :, 0:34]
        xpl[:, :, :, 1:, 1 : W + 1] = src_l[:, :, :, 0:34]
    else:
        xph[:, :, :, 1:, 1 : W + 1] = src_h[:, :, :, 30:64][:, :, :, ::-1]
        xpl[:, :, :, 1:, 1 : W + 1] = src_l[:, :, :, 30:64][:, :, :, ::-1]

    return {
        "xh": xph.reshape(T, 128, 2 * XR * WP),
        "xl": xpl.reshape(T, 128, 2 * XR * WP),
        "xp": np.ascontiguousarray(
            xph[0, :, :, 0:16, :].reshape(128, 2 * 16 * WP)
        ),
        **shared[half],
    }


def kernel(x, w_k, w_q, w_v, w_out, b_out):
    global LAST_RESULT
    from concourse.bass_utils import run_bass_kernel_spmd

    nc = _get_nc()
    shared = _shared_inputs(w_k, w_q, w_v, w_out, b_out)
    xh, xl = _x_splits(x)
    in_maps = [core_inputs(c, x, xh, xl, shared) for c in range(NCORES)]
    res = run_bass_kernel_spmd(
        nc, in_maps, core_ids=list(range(NCORES)), **RUN_KWARGS
    )
    LAST_RESULT = res

    out = np.empty((B, C, H, W), np.float32)
    for c in range(NCORES):
        b, half = divmod(c, 2)
        o = res.results[c]["out"].reshape(C, 32, W)
        if half == 0:
            out[b, :, 0:32, :] = o
        else:
            out[b, :, 32:64, :] = o[:, ::-1, :]
    return np.broadcast_to(out[:, None], (B, T, C, H, W))


# revision 72
# speedup vs baseline: 1.0020x; 1.0020x over previous
"""Trainium2 Bass kernel for the sparse video-attention module.

Model (reference):
    k = conv3x3(x[:, 0], w_k)                     # key from first frame only
    q = conv3x3(x, w_q); v = conv3x3(x, w_v)      # per-frame
    dots[b,t,h,w] = sum_c q[b,t,c,h,w] * k[b,c,h,w]
    attn = softmax_T(dots)
    pooled = sum_t attn[...,t] * v[...,t]         # (B, DH, H, W)
    out = conv3x3(pooled, w_out) + b_out          # identical for every t

Sharding: 8 cores = (batch b in 0..3) x (row half in 0..1). Each core owns 32
output rows of one batch element; all coupling (softmax over T, convs) is
local given the row halo, so there is no inter-core communication. Each core
computes exactly 33 attention rows: its 32 plus the ONE real halo row toward
the seam -- odd cores get their x slice (and conv weights) vertically
flipped host-side so the real halo is always local row 32, and the
out-of-image halo row (pure zeros) is never computed at all. The host
pre-pads/slices inputs per core and re-assembles (un-flipping odd halves) +
broadcasts the output over T at the end.

Precision/engine strategy: all 17 q/k/v conv frames run as fp8e4m3
DoubleRow matmuls (0.5 PE cycles/row, contraction 2x128 channels per
call) with error compensation: x and w are each split hi+lo on the e4m3
grid (host-side) and the conv accumulates xh*wh + xh*wl + xl*wh in PSUM,
dropping only the O(u^2) xl*wl term -- ~0.2% relative error per conv at
3/4 the f32r cycle cost (measured 1.6e-2 final max-rel-err vs the 2e-2
gate; the deterministic inputs make this a fixed, not statistical,
margin). w is pre-scaled by 256 to keep its lo-split out of e4m3
subnormals; the 1/256^2 descale of the q*k product is folded into the
dots ones-vector, and attn rows are scaled by 1/256 via the softmax
normalizer to cancel the 256x the v PSUM carries. The out conv on pooled
stays f32r. A short chain of throwaway ident transposes warms the PE
p-state ramp during the initial DMA wait.

Per-core layout: pixels of the 33 compute rows are flattened row-major into
2112 positions (staged into a zero-padded 17*128 grid for the dots
transposes). Convs are matmuls with C on the contraction dim and 9 spatial
taps accumulated in PSUM; spatial shifts are plain access-pattern offsets
into a zero-padded [128, 2, 35, 66] SBUF image (ktile dim = channel group
for DoubleRow).

Two passes over frames: q-pass computes dots[t, pix] via ones-stationary
M=1 f32r matmuls over qk, PE-transposed to [pixel, t] so the softmax over
T is a free-dim reduction. The v-pass then convolves each frame and
accumulates attn_t * v_t straight from the conv PSUM banks on the Vector
engine, hidden under the next frame's matmuls; attn rows (bf16) reach all
128 partitions via one DRAM-bounced stride-0 broadcast DMA per frame
(frame 0's v conv is drained to SBUF by the Act engine so its PSUM banks
recycle without waiting on that bounce). The final accumulation add
writes a float32r copy of pooled so the out-conv runs at full PE rate,
with the last v conv emitted row-block-outer so the out conv chases the
accumulation chain instead of trailing it.
"""

import sys

import numpy as np

for _p in ("/opt/trn_rl_repo", "/root/.axon_site/_ro/trn_rl_repo"):
    if _p not in sys.path:
        sys.path.insert(0, _p)

import ml_dtypes

E4 = ml_dtypes.float8_e4m3

B, T, C, H, W = 4, 8, 256, 64, 64
DH = 128
NCORES = 8
CR = 33            # compute rows per core (32 own + 1 real halo at the seam)
XR = 35            # x rows per core (locals -1..33: compute rows + conv halo)
WP = W + 2         # zero-padded width
NPIX = CR * W      # 2112 pixels per core
NBLK = 17          # 128-pixel transpose blocks (last one half zero-padded)
SPIX = NBLK * 128  # 2176-wide staging grid for dots/attn transposes
SW = 256.0         # fp8 weight pre-scale (keeps w-lo out of e4m3 subnormals)
# Row blocks over the 33 compute rows. Free dims 448/448/448/384/384 -- all
# >=256 so f32r matmuls run 1 cycle/row, and each fits one PSUM bank.
RB = [(0, 7), (7, 7), (14, 7), (21, 6), (27, 6)]
OUT_RB = [(0, 8), (8, 8), (16, 8), (24, 8)]  # over the 32 output rows

WARMUP_BIG = 12    # PE-ramp warm-up transposes (256-cycle) during DMA lead-in
WARMUP_SMALL = 8   # fine-grained (64-cycle) tail of the warm-up chain

RUN_KWARGS: dict = {}   # extra kwargs for run_bass_kernel_spmd (test hook)
LAST_RESULT = None      # last BassKernelResults (test hook)

_cache: dict = {}


def _build_nc():
    from contextlib import ExitStack

    import concourse.mybir as mybir
    import concourse.tile as tile
    from concourse import bacc
    from concourse.masks import make_identity

    f32 = mybir.dt.float32
    f32r = mybir.dt.float32r
    f8 = mybir.dt.float8e4
    bf16 = mybir.dt.bfloat16
    DR = mybir.MatmulPerfMode.DoubleRow
    AF = mybir.ActivationFunctionType
    X = mybir.AxisListType.X

    nc = bacc.Bacc("TRN2", target_bir_lowering=False)

    xh_d = nc.declare_dram_parameter("xh", [T, 128, 2 * XR * WP], f8, isOutput=False)
    xl_d = nc.declare_dram_parameter("xl", [T, 128, 2 * XR * WP], f8, isOutput=False)
    # duplicate of frame-0 xh rows 0..15: its own tile, so the k conv's
    # leading matmuls aren't gated on the whole frame-0 load (tile
    # dependency tracking is whole-tile, not subtile)
    xp_d = nc.declare_dram_parameter("xp", [128, 2 * 16 * WP], f8, isOutput=False)
    wqh_d = nc.declare_dram_parameter("wqh", [128, 2, 9, 128], f8, isOutput=False)
    wql_d = nc.declare_dram_parameter("wql", [128, 2, 9, 128], f8, isOutput=False)
    wvh_d = nc.declare_dram_parameter("wvh", [128, 2, 9, 128], f8, isOutput=False)
    wvl_d = nc.declare_dram_parameter("wvl", [128, 2, 9, 128], f8, isOutput=False)
    wkh_d = nc.declare_dram_parameter("wkh", [128, 2, 9, 128], f8, isOutput=False)
    wkl_d = nc.declare_dram_parameter("wkl", [128, 2, 9, 128], f8, isOutput=False)
    wo_d = nc.declare_dram_parameter("wo", [128, 9, 256], f32r, isOutput=False)
    bo_d = nc.declare_dram_parameter("bo", [128, 2], f32, isOutput=False)
    out_d = nc.declare_dram_parameter("out", [2, 128, 32 * W], f32, isOutput=True)

    with tile.TileContext(nc) as tc, ExitStack() as ctx:
        singles = ctx.enter_context(tc.tile_pool(name="singles", bufs=1))
        xpool = ctx.enter_context(tc.tile_pool(name="xpool", bufs=5))
        sb = ctx.enter_context(tc.tile_pool(name="sb", bufs=1))
        sm = ctx.enter_context(tc.tile_pool(name="sm", bufs=2))
        qkpool = ctx.enter_context(tc.tile_pool(name="qkpool", bufs=2))
        abpool = ctx.enter_context(tc.tile_pool(name="abpool", bufs=2))
        dtpool = ctx.enter_context(tc.tile_pool(name="dtpool", bufs=2))

        def load_x8(t, split=False):
            # hi/lo e4m3 frame tiles [128, 2(group=ktile), XR*WP]; one DMA
            # per tensor (the per-DMA SP descriptor-gen cost dominates).
            tiles = []
            flats = []
            for tag, dram in (("xh", xh_d), ("xl", xl_d)):
                xt = xpool.tile([128, 2, XR * WP], f8, tag=tag, name=f"{tag}{t}")
                flats.append((xt.rearrange("p g l -> p (g l)"), dram))
                tiles.append(xt.rearrange("p g (r c) -> p g r c", c=WP))
            if split:
                return tiles, flats
            for flat, dram in flats:
                nc.sync.dma_start(out=flat, in_=dram[t])
            return tiles

        # k-conv operands first (wkh + frame-0 x), so the PE starts as early
        # as possible under the serial DMA pipe.
        wkh_sb = singles.tile([128, 2, 9, 128], f8, tag="wkh")
        wkl_sb = singles.tile([128, 2, 9, 128], f8, tag="wkl")
        nc.sync.dma_start(out=wkh_sb, in_=wkh_d[:])
        # frame 0 arrives in k-conv consumption order: the rows-0..15 prefix
        # first (feeds the front-run matmuls), then the full frame, then wkl
        # and xl -- needed only from the conv's second/third term on.
        xp0 = singles.tile([128, 2, 16, WP], f8, tag="xp0")
        nc.sync.dma_start(out=xp0.rearrange("p g r c -> p (g r c)"), in_=xp_d[:])
        x8_0, _fl0 = load_x8(0, split=True)
        _xh3 = _fl0[0][0].rearrange("p (g l) -> p g l", g=2)
        _sh3 = _fl0[0][1][0].rearrange("p (g l) -> p g l", g=2)
        L0 = XR * WP
        for h in range(2):
            cols = slice(h * L0 // 2, (h + 1) * L0 // 2)
            nc.sync.dma_start(out=_xh3[:, :, cols], in_=_sh3[:, :, cols])
        nc.sync.dma_start(out=wkl_sb, in_=wkl_d[:])
        nc.sync.dma_start(out=_fl0[1][0], in_=_fl0[1][1][0])
        wqh_sb = singles.tile([128, 2, 9, 128], f8, tag="wqh")
        wql_sb = singles.tile([128, 2, 9, 128], f8, tag="wql")
        wvh_sb = singles.tile([128, 2, 9, 128], f8, tag="wvh")
        wvl_sb = singles.tile([128, 2, 9, 128], f8, tag="wvl")
        wo_sb = singles.tile([128, 9, 256], f32r, tag="wo")
        bo_sb = singles.tile([128, 2], f32, tag="bo")
        nc.sync.dma_start(out=bo_sb, in_=bo_d[:])
        nc.sync.dma_start(out=wqh_sb, in_=wqh_d[:])
        nc.sync.dma_start(out=wql_sb, in_=wql_d[:])
        nc.sync.dma_start(out=wvh_sb, in_=wvh_d[:])
        nc.sync.dma_start(out=wvl_sb, in_=wvl_d[:])
        nc.sync.dma_start(out=wo_sb, in_=wo_d[:])

        # The PE warm-up chain reads ident BEFORE it is initialized (the
        # transposed garbage lands in a PSUM region that is start=True
        # overwritten later): the memset + affine_select that build the real
        # identity run after the warm-up reads, still long before the first
        # true transpose use at the dots gather.
        ident = singles.tile([128, 128], f32, tag="ident")
        ident_bf = singles.tile([128, 128], bf16, tag="ident_bf")
        ones_col = singles.tile([128, 1], f32, tag="ones")
        # q and k PSUMs both carry the SW weight scale; the ones column
        # descales their product in the dots reduction
        nc.vector.memset(ones_col, 1.0 / (SW * SW))
        ones_r = singles.tile([128, 1], f32r, tag="ones_r")
        nc.vector.tensor_copy(ones_r, ones_col)
        csw = singles.tile([128, 1], f32, tag="csw")
        nc.vector.memset(csw, 1.0 / SW)       # attn descale (cancels v scale)
        eps_sb = singles.tile([128, 1], f32, tag="eps")
        nc.vector.memset(eps_sb, 1e-30)

        def conv3x3_f8(psums, x8, wh_sb, wl_sb, full=False, r_outer=False,
                       schedule=None):
            # compensated fp8 conv: xh*wh + xh*wl + xl*wh (+ xl*wl when
            # full=True), DoubleRow over both channel groups (ktile dim) in
            # each call. r_outer finishes PSUM banks in row-block order so
            # downstream per-block consumers can chase the conv; schedule
            # overrides the (term, tap, block) emission order entirely.
            xh4, xl4 = x8
            terms = [(xh4, wh_sb), (xh4, wl_sb), (xl4, wh_sb)]
            if full:
                terms.append((xl4, wl_sb))
            nt = len(terms)

            def mm(ti, j, r):
                xt4, w_sb = terms[ti]
                ky, kx = divmod(j, 3)
                R0, nr = RB[r]
                nc.tensor.matmul(
                    psums[r][:, : nr * W],
                    w_sb[:, :, j, :],
                    xt4[:, :, R0 + ky : R0 + ky + nr, kx : kx + W],
                    start=(ti == 0 and j == 0),
                    stop=(ti == nt - 1 and j == 8),
                    perf_mode=DR,
                )

            if schedule is None:
                if r_outer:
                    schedule = [
                        (ti, j, r)
                        for r in range(len(RB))
                        for ti in range(nt)
                        for j in range(9)
                    ]
                else:
                    schedule = [
                        (ti, j, r)
                        for ti in range(nt)
                        for j in range(9)
                        for r in range(len(RB))
                    ]
            for ti, j, r in schedule:
                mm(ti, j, r)

        dpool = ctx.enter_context(tc.tile_pool(name="dpool", bufs=1, space="DRAM"))
        attnT_dram = dpool.tile([8, SPIX], bf16, tag="attnTd")

        k_sb = sb.tile([128, NPIX], f32, tag="k")
        attnT_sb = sb.tile([8, SPIX], bf16, tag="attnT")
        # pooled rows 0..33 = locals -1..32; row 0 is the zero out-of-image
        # halo (memset once), computed locals land at tile row +1
        pooled = sb.tile([128, CR + 1, WP], f32, tag="pooled")
        pooled_r = sb.tile([128, CR + 1, WP], f32r, tag="pooled_r")
        out_sb = sb.tile([128, 2, 32 * W], f32, tag="out")

        psc = ctx.enter_context(tc.tile_pool(name="psc", bufs=6, space="PSUM"))

        def vconv(t, x8):
            vps = [
                psc.tile([128, 512], f32, tag="cv", name=f"vps{t}_{r}")
                for r in range(len(RB))
            ]
            # the last frame releases its PSUM banks in row-block order so
            # the final vapply chain and the out conv can chase it
            conv3x3_f8(vps, x8, wvh_sb, wvl_sb, r_outer=(t == T - 1))
            return vps

        def vapply(t, vsrc):
            # pooled += attn_t (broadcast over channels) * v_t; vsrc(r) gives
            # the v rows for block r (conv PSUM bank, or SBUF for frame 0).
            # attn rows carry 1/SW so the product is true-scale. The last
            # frame's add writes the f32r copy the out-conv consumes.
            # One whole-frame broadcast DMA: per-DMA SP descriptor-gen
            # (~650ns) dominates small transfers.
            abf = abpool.tile([128, SPIX], bf16, tag="ab", name=f"ab{t}")
            nc.sync.dma_start(
                out=abf,
                in_=attnT_dram[t : t + 1, :].to_broadcast((128, SPIX)),
            )
            for r, (R0, nr) in enumerate(RB):
                rows = slice(R0 + 1, R0 + 1 + nr)
                cols = slice(R0 * W, (R0 + nr) * W)
                ab = abf[:, R0 * W :]
                if t == 0:
                    nc.vector.tensor_mul(
                        pooled[:, rows, 1 : W + 1],
                        vsrc(r).rearrange("p (r c) -> p r c", c=W),
                        ab[:, : nr * W].rearrange("p (r c) -> p r c", c=W),
                    )
                    continue
                u = qkpool.tile([128, NPIX], f32, tag="qk", name=f"u{t}_{r}")
                nc.vector.tensor_mul(u[:, cols], vsrc(r), ab[:, : nr * W])
                dst = pooled_r if t == T - 1 else pooled
                nc.vector.tensor_add(
                    dst[:, rows, 1 : W + 1],
                    pooled[:, rows, 1 : W + 1],
                    u[:, cols].rearrange("p (r c) -> p r c", c=W),
                )

        with (
            tc.tile_pool(name="psd", bufs=1, space="PSUM") as psd,
            tc.tile_pool(name="psdd", bufs=1, space="PSUM") as psdd,
        ):
            dots_ps = psd.tile([128, NBLK * 8], f32, tag="dots")
            # PE warm-up: the cost model ramps the PE clock (0.65 -> 1.2 ->
            # 2.4 GHz) over the first ~3us of continuous busyness. Burn the
            # ramp on throwaway ident transposes during the initial DMA wait
            # instead of on the k conv. Tuned to drain right as x0 lands.
            for i in range(WARMUP_BIG):
                nc.tensor.transpose(dots_ps[:, 0:128], ident, ident)
            for i in range(WARMUP_SMALL):
                nc.tensor.transpose(
                    dots_ps[:32, 0:32], ident[:32, :32], ident[:32, :32]
                )
            make_identity(nc, ident)
            nc.vector.tensor_copy(ident_bf, ident)
            dots_sb = sb.tile([8, SPIX], f32, tag="dsb")
            # the staging grid's zero tail (pixels NPIX..SPIX) keeps the
            # padded transpose block NaN-free
            nc.vector.memset(dots_sb[:, NPIX:], 0.0)

            # ---- phase 1: k = conv(x[0], w_k), fp8 ----
            kps = [
                psc.tile([128, 512], f32, tag="cv", name=f"kps{r}")
                for r in range(len(RB))
            ]
            # front-run: first-term matmuls for row blocks 0-1 read the
            # dedicated rows-0..15 prefix tile, so they start on ~3.6us of
            # DMA instead of waiting out the full frame-0 load
            for j in range(9):
                ky, kx = divmod(j, 3)
                for r in (0, 1):
                    R0, nr = RB[r]
                    nc.tensor.matmul(
                        kps[r][:, : nr * W],
                        wkh_sb[:, :, j, :],
                        xp0[:, :, R0 + ky : R0 + ky + nr, kx : kx + W],
                        start=(j == 0),
                        stop=False,
                        perf_mode=DR,
                    )
            ksched = (
                [(0, j, r) for j in range(9) for r in (2, 3, 4)]
                + [(ti, j, r) for ti in (1, 2) for j in range(9) for r in range(5)]
            )
            conv3x3_f8(kps, x8_0, wkh_sb, wkl_sb, schedule=ksched)
            for r, (R0, nr) in enumerate(RB):
                nc.scalar.activation(
                    k_sb[:, R0 * W : (R0 + nr) * W], kps[r][:, : nr * W], AF.Copy
                )

            # ---- phase 2: per frame q conv (fp8) + dots ----
            for t in range(T):
                x8 = x8_0 if t == 0 else load_x8(t)
                qps = [
                    psc.tile([128, 512], f32, tag="cv", name=f"qps{t}_{r}")
                    for r in range(len(RB))
                ]
                conv3x3_f8(qps, x8, wqh_sb, wql_sb)
                qk = qkpool.tile([128, NPIX], f32r, tag="qk", name=f"qk{t}")
                dtmp = dtpool.tile([1, NPIX], f32, tag="dtmp", name=f"dt{t}")
                for r, (R0, nr) in enumerate(RB):
                    cols = slice(R0 * W, (R0 + nr) * W)
                    nc.vector.tensor_mul(qk[:, cols], qps[r][:, : nr * W], k_sb[:, cols])
                    # dots[t, pix] = (1/SW^2) * sum_c qk[c, pix]: the ones
                    # column descales the weight scale both convs carry
                    dps = psdd.tile([1, 512], f32, tag="dd", name=f"dd{t}_{r}")
                    nc.tensor.matmul(
                        dps[:, : nr * W],
                        ones_r[:, 0:1],
                        qk[:, cols],
                        start=True,
                        stop=True,
                    )
                    nc.scalar.activation(dtmp[:, cols], dps[:, : nr * W], AF.Copy)
                    if t == T - 1:
                        # last frame: per-block staging DMAs so the gather
                        # transposes (waiting on row 7) release incrementally
                        nc.sync.dma_start(
                            out=dots_sb[t : t + 1, cols], in_=dtmp[:, cols]
                        )
                if t < T - 1:
                    # one DMA per frame into the [t, pix] staging row
                    nc.sync.dma_start(out=dots_sb[t : t + 1, :NPIX], in_=dtmp)

            # gather dots into [pixel, t] layout for the softmax
            for i in range(NBLK):
                nc.tensor.transpose(
                    dots_ps[:, i * 8 : (i + 1) * 8],
                    dots_sb[:, i * 128 : (i + 1) * 128],
                    ident[:8, :8],
                )

            # v conv for frame 0 keeps the PE busy through the softmax below;
            # reuses the still-resident x0 fp8 tiles (no reload). Drain it to
            # SBUF on the idle Act engine right away: vapply(0) sits behind
            # the attn DRAM-bounce, and holding the PSUM banks that long
            # would stall vconv(1). Prefetch the first v-pass frames NOW:
            # DMAs issued later queue behind the attn-dependent attnT
            # transfers (head-of-line) and would stall the v-pass.
            vps0 = vconv(0, x8_0)
            v0_sb = sb.tile([128, NPIX], f32, tag="v0")
            for r, (R0, nr) in enumerate(RB):
                nc.scalar.activation(
                    v0_sb[:, R0 * W : (R0 + nr) * W], vps0[r][:, : nr * W], AF.Copy
                )
            vload = {t: load_x8(t) for t in (1, 2)}

            # ---- softmax over t (free dim) ----
            dots3 = dots_ps.rearrange("p (i t) -> p i t", t=8)
            nmax = sm.tile([128, NBLK], f32, tag="nmax")
            nc.vector.reduce_max(out=nmax, in_=dots3, axis=X, negate=True)
            dm = sm.tile([128, NBLK, 8], f32, tag="dm")
            nc.vector.tensor_add(
                dm, dots3, nmax[:, :, None].to_broadcast((128, NBLK, 8))
            )
            nc.scalar.activation(dm, dm, AF.Exp)
            ssum = sm.tile([128, NBLK], f32, tag="ssum")
            nc.vector.reduce_sum(out=ssum, in_=dm, axis=X)
            nc.scalar.add(ssum, ssum, eps_sb[:])
            rs = sm.tile([128, NBLK], f32, tag="rs")
            nc.vector.reciprocal(rs, ssum)
            # fold the v-path descale into the normalizer: attn rows = true/SW
            nc.vector.tensor_mul(rs, rs, csw[:, 0:1].to_broadcast((128, NBLK)))
            attn = sm.tile([128, NBLK, 8], bf16, tag="attn")
            nc.vector.tensor_mul(
                attn, dm, rs[:, :, None].to_broadcast((128, NBLK, 8))
            )

        # ---- transpose attn to [t, pixel], bounce via DRAM for broadcast ----
        with tc.tile_pool(name="pst", bufs=2, space="PSUM") as pst:
            for ci in range((NBLK + 3) // 4):
                blocks = range(4 * ci, min(4 * ci + 4, NBLK))
                tp = pst.tile([8, 512], bf16, tag="attnT_ps", name=f"tp{ci}")
                for ib, i in enumerate(blocks):
                    nc.tensor.transpose(
                        tp[:, ib * 128 : (ib + 1) * 128], attn[:, i, :], ident_bf
                    )
                n = len(blocks) * 128
                c0 = 4 * ci * 128
                nc.vector.tensor_copy(attnT_sb[:, c0 : c0 + n], tp[:, :n])
            nc.sync.dma_start(out=attnT_dram[:], in_=attnT_sb)

        # ---- phase 3: v convs (fp8) with attn-weighted accumulation ----
        nc.vector.memset(pooled_r[:, 0:1].bitcast(f32), 0.0)
        nc.vector.memset(pooled_r[:, :, 0:1].bitcast(f32), 0.0)
        nc.vector.memset(pooled_r[:, :, W + 1 : W + 2].bitcast(f32), 0.0)
        vapply(0, lambda r: v0_sb[:, RB[r][0] * W : (RB[r][0] + RB[r][1]) * W])
        for t in range(1, T):
            x8 = vload.pop(t, None) or load_x8(t)
            if t + 2 < T and t + 2 not in vload:
                vload[t + 2] = load_x8(t + 2)
            vps = vconv(t, x8)
            vapply(t, lambda r, vps=vps: vps[r][:, : RB[r][1] * W])

        # ---- phase 4: out = conv(pooled, w_out) + b, f32r ----
        with tc.tile_pool(name="pso", bufs=2, space="PSUM") as pso:
            for R0o, nr in OUT_RB:
                for g in range(2):
                    # the very last group is the exit critical path: run it
                    # as two independent row-half pieces so its store chain
                    # pipelines with its matmuls
                    pieces = 2 if (R0o, g) == (OUT_RB[-1][0], 1) else 1
                    pnr = nr // pieces
                    for ci in range(pieces):
                        Rp = R0o + ci * pnr
                        op = pso.tile(
                            [128, 512], f32, tag="out_ps", name=f"op{Rp}_{g}"
                        )
                        for j in range(9):
                            ky, kx = divmod(j, 3)
                            nc.tensor.matmul(
                                op[:, : pnr * W],
                                wo_sb[:, j, g * 128 : (g + 1) * 128],
                                pooled_r[:, Rp + ky : Rp + ky + pnr, kx : kx + W],
                                start=(j == 0),
                                stop=(j == 8),
                            )
                        nc.scalar.add(
                            out_sb[:, g, Rp * W : (Rp + pnr) * W],
                            op[:, : pnr * W],
                            bo_sb[:, g : g + 1],
                        )
                        nc.sync.dma_start(
                            out=out_d[g, :, Rp * W : (Rp + pnr) * W],
                            in_=out_sb[:, g, Rp * W : (Rp + pnr) * W],
                        )

    nc.compile()
    return nc


def _get_nc():
    if "nc" not in _cache:
        _cache["nc"] = _build_nc()
    return _cache["nc"]


def _round_f32r(a):
    """Round fp32 to the FP32r grid (e8m11 in the top 20 bits, RNE)."""
    u = np.ascontiguousarray(a, np.float32).view(np.uint32).copy()
    u += np.uint32(0x7FF) + ((u >> np.uint32(12)) & np.uint32(1))
    u &= np.uint32(0xFFFFF000)
    return u.view(np.float32)


def _split8(a):
    """hi/lo e4m3 split: a ~= hi + lo with ~u^2 residual."""
    a = np.ascontiguousarray(a, np.float32)
    hi = a.astype(E4)
    lo = (a - hi.astype(np.float32)).astype(E4)
    return hi, lo


def _shared_inputs(w_k, w_q, w_v, w_out, b_out):
    def conv_lhst(w, scale=1.0):  # (co=128, ci=256, 3, 3) -> (ci128, g, j, co)
        return np.ascontiguousarray(
            np.asarray(w, np.float32)
            .reshape(128, 2, 128, 3, 3)
            .transpose(2, 1, 3, 4, 0)
            .reshape(128, 2, 9, 128)
        ) * scale

    bo = np.ascontiguousarray(np.asarray(b_out, np.float32).reshape(2, 128).T)

    def one(flip):
        # odd cores see a vertically flipped image: flip the kernel rows too
        def f(w):
            w = np.asarray(w, np.float32)
            return w[:, :, ::-1, :] if flip else w

        wqh, wql = _split8(conv_lhst(f(w_q), SW))
        wvh, wvl = _split8(conv_lhst(f(w_v), SW))
        wkh, wkl = _split8(conv_lhst(f(w_k), SW))
        wo = np.ascontiguousarray(  # (co=256, dh=128, 3, 3) -> (dh, j, co)
            f(w_out).transpose(1, 2, 3, 0).reshape(128, 9, 256)
        )
        return {
            "wqh": wqh, "wql": wql, "wvh": wvh, "wvl": wvl,
            "wkh": wkh, "wkl": wkl,
            "wo": _round_f32r(wo),
            "bo": bo,
        }

    return [one(False), one(True)]


def _x_splits(x):
    xf = np.ascontiguousarray(x, np.float32)
    xh = xf.astype(E4)
    xl = (xf - xh.astype(np.float32)).astype(E4)
    return xh, xl


def core_inputs(c, x, xh, xl, shared):
    b, half = divmod(c, 2)
    # canonical core layout: x tile rows 0..34 = locals -1..33, computed
    # attention rows = locals 0..32 (local 32 is the real seam halo), out
    # rows = locals 0..31. half 0: local == global (tile rows 1..34 <-
    # globals 0..33, row 0 zero). half 1: local L == global 63-L, i.e. the
    # slice is vertically flipped (tile rows 1..34 <- globals 63..30
    # reversed, row 0 = global 64 = zero).
    xph = np.zeros((T, 128, 2, XR, WP), E4)
    xpl = np.zeros((T, 128, 2, XR, WP), E4)
    src_h = xh[b].reshape(T, 2, 128, H, W).transpose(0, 2, 1, 3, 4)
    src_l = xl[b].reshape(T, 2, 128, H, W).transpose(0, 2, 1, 3, 4)
    if half == 0:
        xph[:, :, :, 1:, 1 : W + 1] = src_h[:, :, :, 0:34]
        xpl[:, :, :, 1:, 1 : W + 1] = src_l[:, :, :, 0:34]
    else:
        xph[:, :, :, 1:, 1 : W + 1] = src_h[:, :, :, 30:64][:, :, :, ::-1]
        xpl[:, :, :, 1:, 1 : W + 1] = src_l[:, :, :, 30:64][:, :, :, ::-1]

    return {
        "xh": xph.reshape(T, 128, 2 * XR * WP),
        "xl": xpl.reshape(T, 128, 2 * XR * WP),
        "xp": np.ascontiguousarray(
            xph[0, :, :, 0:16, :].reshape(128, 2 * 16 * WP)
        ),
        **shared[half],
    }


def kernel(x, w_k, w_q, w_v, w_out, b_out):
    global LAST_RESULT
    from concourse.bass_utils import run_bass_kernel_spmd

    nc = _get_nc()
    shared = _shared_inputs(w_k, w_q, w_v, w_out, b_out)
    xh, xl = _x_splits(x)
    in_maps = [core_inputs(c, x, xh, xl, shared) for c in range(NCORES)]
    res = run_bass_kernel_spmd(
        nc, in_maps, core_ids=list(range(NCORES)), **RUN_KWARGS
    )
    LAST_RESULT = res

    out = np.empty((B, C, H, W), np.float32)
    for c in range(NCORES):
        b, half = divmod(c, 2)
        o = res.results[c]["out"].reshape(C, 32, W)
        if half == 0:
            out[b, :, 0:32, :] = o
        else:
            out[b, :, 32:64, :] = o[:, ::-1, :]
    return np.broadcast_to(out[:, None], (B, T, C, H, W))
